# revision 32
# baseline (speedup 1.0000x reference)
"""Trainium2 Bass kernel for the CoAttention DNS/Image module.

Math notes (exact algebraic simplification of the reference):
  scores1[b,r,s] = s_img[b,r] + s_dns[b,s] + b_att1 ; softmax over s.
  The per-row constant s_img[b,r] (and b_att1) cancels in the softmax, so
  a1[b,r,:] == softmax(s_dns[b,:]) for every r. Hence
      att_dns[b,r,:] = softmax(s_dns[b]) @ dns[b]          (same for all r)
  Similarly scores2's softmax over j kills t_dns[b,i] and b_att2, so
      att_img[b,i,:] = softmax(t_img[b]) @ img[b]          (same for all i)
  The remaining work per batch item:
      s_dns[s] = tanh(dns[b] @ W_dns1.T + b_dns1) @ w_att1[H:]
      t_img[j] = tanh(img[b] @ W_img2.T + b_img2) @ w_att2[H:]
  plus two softmaxes and two weighted sums.  Since every output row is
  identical, the device only produces the per-item attended vector v
  (B, H) per side; the host broadcasts to (B, R, H).

Device design (per core, 8 batch items, bf16 matmul datapath):
  - inputs staged host-side in final SBUF layout (transposed, hc-chunked,
    bf16) so every DMA is a contiguous partition-line stream.
  - projection: oc x hc matmul tiling, psum accumulate over hc.
  - logits: the w-dot uses a replicated-column stationary operand
    (128 copies of the w chunk) so the logit row comes out of the PE
    broadcast across all 128 partitions; softmax then runs on wide
    (128, n) tiles only.
  - attended vector: DVE scalar_tensor_tensor (fused multiply + free-axis
    accumulate; tensor_tensor_reduce crashes this HW) of the already-loaded
    transposed activations against the broadcast UNNORMALIZED exp weights,
    accumulated per (item, h-chunk) into a per-core (128, 64) tile and
    stored with the softmax normalizers Z; the host divides u/Z and
    broadcasts rows.
"""

import sys

import numpy as np

try:
    import concourse  # noqa: F401
except ImportError:  # fresh environment: fall back to the repo path
    sys.path.insert(0, "/opt/trn_rl_repo")

B, S, R, H = 64, 256, 196, 1024
NCORES = 8
BPC = B // NCORES        # batch items per core = 8
PAIRS = BPC // 2         # items are processed in pairs = 4
HC = H // 128            # 8 chunks of the feature dim
ND = 2 * S               # dns pair free width  = 512
NG = 2 * R               # img pair free width  = 392

_CACHE = {}


def _build_program(loop_n=0, probe=None):
    """probe: None = full kernel; 'pe' = drop exp/reduce/v-stores (PE+Act+DMA
    only); 'proj' = projection matmuls + input DMAs only."""
    import contextlib
    from contextlib import ExitStack

    import concourse.bacc as bacc
    import concourse.tile as tile
    from concourse import mybir

    f32 = mybir.dt.float32
    bf16 = mybir.dt.bfloat16
    Act = mybir.ActivationFunctionType
    Alu = mybir.AluOpType

    nc = bacc.Bacc("TRN2", target_bir_lowering=False, debug=False)

    dtT = nc.dram_tensor("dtT", (PAIRS, 128, HC * ND), bf16, kind="ExternalInput").ap()
    gtT = nc.dram_tensor("gtT", (PAIRS, 128, HC * NG), bf16, kind="ExternalInput").ap()
    # weights chunked by OUTPUT block (oc): tile oc holds all contraction
    # chunks for that block, so the first oc iteration only needs 256 KB
    # of weights on-chip instead of the full 2 MB.
    w1d = nc.dram_tensor("w1d", (HC, 128, HC * 128), bf16,
                         kind="ExternalInput").ap()
    w4d = nc.dram_tensor("w4d", (HC, 128, HC * 128), bf16,
                         kind="ExternalInput").ap()
    wr1 = nc.dram_tensor("wr1", (128, HC * 128), bf16, kind="ExternalInput").ap()
    wr4 = nc.dram_tensor("wr4", (128, HC * 128), bf16, kind="ExternalInput").ap()
    bc1 = nc.dram_tensor("bc1", (128, HC), f32, kind="ExternalInput").ap()
    bc4 = nc.dram_tensor("bc4", (128, HC), f32, kind="ExternalInput").ap()

    vD = nc.dram_tensor("v_dns", (128, BPC * HC), f32, kind="ExternalOutput").ap()
    # img attended vectors (cols 0:64) + softmax normalizers Z for both
    # sides (cols 64:80) in one store
    vG = nc.dram_tensor("v_img_z", (128, BPC * HC + 2 * BPC), f32,
                        kind="ExternalOutput").ap()

    with tile.TileContext(nc) as tc, ExitStack() as ctx:
        consts = ctx.enter_context(tc.tile_pool(name="consts", bufs=1))
        acts = ctx.enter_context(tc.tile_pool(name="acts", bufs=2))
        tts = ctx.enter_context(tc.tile_pool(name="tts", bufs=3))
        sm = ctx.enter_context(tc.tile_pool(name="sm", bufs=2))
        scr = ctx.enter_context(tc.tile_pool(name="scr", bufs=2))
        vout = ctx.enter_context(tc.tile_pool(name="vout", bufs=1))
        pproj = ctx.enter_context(tc.tile_pool(name="pproj", bufs=4, space="PSUM"))
        psrow = ctx.enter_context(tc.tile_pool(name="psrow", bufs=3, space="PSUM"))

        # --- tile allocations (persistent / tag-cycled) ---
        dt0_c = [consts.tile([128, ND], bf16, name=f"dt0c{h}") for h in range(HC)]
        w1_c = [consts.tile([128, HC * 128], bf16, name=f"w1o{o}")
                for o in range(HC)]
        b1_sb = consts.tile([128, HC], f32, name="b1_sb")
        wr1_sb = consts.tile([128, HC * 128], bf16, name="wr1_sb")
        gt0_c = [consts.tile([128, NG], bf16, name=f"gt0c{h}") for h in range(HC)]
        w4_c = [consts.tile([128, HC * 128], bf16, name=f"w4o{o}")
                for o in range(HC)]
        b4_sb = consts.tile([128, HC], f32, name="b4_sb")
        wr4_sb = consts.tile([128, HC * 128], bf16, name="wr4_sb")
        dgt = {}  # (pr, side) -> activation tile (pairs >= 1)
        for pr in (1, 2, 3):
            dgt[pr, 0] = acts.tile([128, HC * ND], bf16, tag="dt", name=f"dt{pr}")
            dgt[pr, 1] = acts.tile([128, HC * NG], bf16, tag="gt", name=f"gt{pr}")

        vd_sb = vout.tile([128, BPC * HC], f32, name="vd_sb")
        vg_sb = vout.tile([128, BPC * HC + 2 * BPC], f32, name="vg_sb")
        z_col = lambda side, it: vg_sb[:, BPC * HC + side * BPC + it:
                                       BPC * HC + side * BPC + it + 1]

        if probe is not None:
            nc.vector.memset(vd_sb, 0)
            nc.vector.memset(vg_sb, 0)

        # preload both activation tables (Tanh, Exp) during the DMA ramp so
        # no mid-kernel table switch stalls the scalar engine
        warm = vout.tile([128, 1], f32, name="warm")
        nc.vector.memset(warm, 0)
        nc.scalar.activation(out=warm, in_=warm, func=Act.Tanh)
        nc.scalar.activation(out=warm, in_=warm, func=Act.Exp)

        if loop_n:
            # Timing builds: weights load once before the hardware loop so
            # the per-iteration delta measures the steady state, not
            # artificial weight reloads.
            nc.scalar.dma_start(out=w1_c[0], in_=w1d[0])
            nc.scalar.dma_start(out=b1_sb, in_=bc1)
            nc.scalar.dma_start(out=wr1_sb, in_=wr1)
            for o in range(1, HC):
                nc.scalar.dma_start(out=w1_c[o], in_=w1d[o])
            nc.scalar.dma_start(out=w4_c[0], in_=w4d[0])
            nc.scalar.dma_start(out=b4_sb, in_=bc4)
            nc.scalar.dma_start(out=wr4_sb, in_=wr4)
            for o in range(1, HC):
                nc.scalar.dma_start(out=w4_c[o], in_=w4d[o])

        loop_cm = (tc.For_i(0, loop_n, 1, hint_engines=(mybir.EngineType.PE,))
                   if loop_n else contextlib.nullcontext())
        with loop_cm:
            # DMAs issued in consumption order on the SP ring.  Processing
            # order puts the dns sides of pairs 0-1 first so the img-side
            # weights have slack to stream in behind them.
            def wdma(out, in_):
                if not loop_n:
                    nc.sync.dma_start(out=out, in_=in_)

            wdma(w1_c[0], w1d[0])
            for h in range(HC):
                nc.sync.dma_start(out=dt0_c[h], in_=dtT[0, :, h * ND:(h + 1) * ND])
            wdma(b1_sb, bc1)
            for o in range(1, HC):
                wdma(w1_c[o], w1d[o])
            wdma(wr1_sb, wr1)
            nc.sync.dma_start(out=dgt[1, 0], in_=dtT[1])
            wdma(w4_c[0], w4d[0])
            for h in range(HC):
                nc.sync.dma_start(out=gt0_c[h], in_=gtT[0, :, h * NG:(h + 1) * NG])
            wdma(b4_sb, bc4)
            for o in range(1, HC):
                wdma(w4_c[o], w4d[o])
            wdma(wr4_sb, wr4)
            nc.sync.dma_start(out=dgt[1, 1], in_=gtT[1])
            for pr in (2, 3):
                nc.sync.dma_start(out=dgt[pr, 0], in_=dtT[pr])
                nc.sync.dma_start(out=dgt[pr, 1], in_=gtT[pr])

            for pr, side in ((0, 0), (1, 0), (0, 1), (1, 1),
                             (2, 0), (2, 1), (3, 0), (3, 1)):
                    if side == 0:
                        w_c, wr_sb, b_sb, n, ns, v_sb = (
                            w1_c, wr1_sb, b1_sb, ND, S, vd_sb)
                        a_c = ((lambda h: dt0_c[h]) if pr == 0 else
                               (lambda h, t=dgt[pr, 0]: t[:, h * ND:(h + 1) * ND]))
                    else:
                        w_c, wr_sb, b_sb, n, ns, v_sb = (
                            w4_c, wr4_sb, b4_sb, NG, R, vg_sb)
                        a_c = ((lambda h: gt0_c[h]) if pr == 0 else
                               (lambda h, t=dgt[pr, 1]: t[:, h * NG:(h + 1) * NG]))

                    # The final pair-side is processed per item so the last
                    # item's softmax+reduce chain overlaps the other item's
                    # projection instead of trailing the whole kernel.
                    groups = [(0, 1)] if (pr, side) != (3, 1) else [(0,), (1,)]
                    for js in groups:
                        gw = len(js) * ns       # group width
                        g0 = js[0] * ns         # group offset in the pair
                        # logits row, broadcast across all 128 partitions.
                        # The logit matmul for block oc is issued after the
                        # NEXT projection block so the PE never waits on the
                        # tanh latency (tanh(oc) overlaps pj(oc+1)).
                        srow = psrow.tile([128, gw], f32, tag="srow",
                                          name=f"sr{pr}_{side}_{js[0]}")
                        prev_tt = None
                        for oc in range(HC):
                            pj = pproj.tile([128, gw], f32, tag="pj",
                                            name=f"pj{pr}_{side}_{js[0]}_{oc}")
                            for hc in range(HC):
                                nc.tensor.matmul(
                                    pj,
                                    lhsT=w_c[oc][:, hc * 128:(hc + 1) * 128],
                                    rhs=a_c(hc)[:, g0:g0 + gw],
                                    start=(hc == 0),
                                    stop=(hc == HC - 1),
                                )
                            if probe == "proj":
                                if oc == HC - 1:
                                    tt = tts.tile([128, gw], bf16, tag="tt",
                                                  name=f"tt{pr}_{side}_{js[0]}")
                                    nc.scalar.activation(
                                        out=tt, in_=pj, func=Act.Tanh,
                                        bias=b_sb[:, oc:oc + 1], scale=1.0)
                                continue
                            if prev_tt is not None:
                                nc.tensor.matmul(
                                    srow,
                                    lhsT=wr_sb[:, (oc - 1) * 128:oc * 128],
                                    rhs=prev_tt,
                                    start=(oc - 1 == 0),
                                    stop=False,
                                )
                            tt = tts.tile([128, gw], bf16, tag="tt",
                                          name=f"tt{pr}_{side}_{js[0]}_{oc}")
                            nc.scalar.activation(
                                out=tt, in_=pj, func=Act.Tanh,
                                bias=b_sb[:, oc:oc + 1], scale=1.0,
                            )
                            prev_tt = tt
                        if probe == "proj":
                            continue
                        nc.tensor.matmul(
                            srow,
                            lhsT=wr_sb[:, (HC - 1) * 128:HC * 128],
                            rhs=prev_tt,
                            start=False,
                            stop=True,
                        )

                        if probe == "pe":
                            continue
                        # exp on wide tiles; logits are bounded (|s| <=
                        # sum|w| ~ 16) so max-subtraction is unneeded in
                        # fp32.  The softmax normalizer Z is shipped to the
                        # host, which divides the unnormalized sums.
                        e_b = sm.tile([128, gw], bf16, tag="eb",
                                      name=f"eb{pr}_{side}_{js[0]}")
                        for j in js:
                            it = 2 * pr + j
                            o = j * ns - g0
                            nc.scalar.activation(
                                out=e_b[:, o:o + ns],
                                in_=srow[:, o:o + ns],
                                func=Act.Exp,
                                accum_out=z_col(side, it),
                            )

                        # unnormalized attended vector u = sum_s e_s * x_s
                        # via fused multiply+reduce on the DVE
                        for j in js:
                            it = 2 * pr + j
                            o = j * ns - g0
                            for hc in range(HC):
                                sc_t = scr.tile([128, ns], bf16, tag="scr",
                                                name=f"sc{pr}_{side}_{j}_{hc}")
                                nc.vector.scalar_tensor_tensor(
                                    out=sc_t,
                                    in0=a_c(hc)[:, j * ns:(j + 1) * ns],
                                    scalar=1.0,
                                    in1=e_b[:, o:o + ns],
                                    op0=Alu.mult, op1=Alu.mult,
                                    accum_out=v_sb[:, it * HC + hc:
                                                   it * HC + hc + 1],
                                )
                    if probe is None and (pr, side) == (3, 0):
                        # all dns-side results done; overlap the store with
                        # the final img side
                        nc.scalar.dma_start(out=vD, in_=vd_sb)
                    if probe is None and (pr, side) == (2, 1):
                        # img items 0-5 done; overlap their store too
                        nc.scalar.dma_start(out=vG[:, :6 * HC],
                                            in_=vg_sb[:, :6 * HC])

        if probe is not None:
            # keep the output tensors bound: store whatever is in the tiles
            nc.sync.dma_start(out=vD, in_=vd_sb)
        nc.sync.dma_start(out=vG[:, 6 * HC:], in_=vg_sb[:, 6 * HC:])

    nc.compile()
    return nc


def _get_program(loop_n=0):
    key = ("prog", loop_n)
    if key not in _CACHE:
        _CACHE[key] = _build_program(loop_n=loop_n)
    return _CACHE[key]


def _bf16(x):
    import ml_dtypes
    return np.asarray(x, np.float32).astype(ml_dtypes.bfloat16)


def _prepare_in_maps(dns_feature, img_features, W_dns1, b_dns1, W_img2, b_img2,
                     w_att1, w_att2):
    dns_bf = _bf16(dns_feature)
    img_bf = _bf16(img_features)
    # (c, pr, j, s, hc, p) -> (c, pr, p, hc, j, s)
    dtT = np.ascontiguousarray(
        dns_bf.reshape(NCORES, PAIRS, 2, S, HC, 128)
        .transpose(0, 1, 5, 4, 2, 3)
        .reshape(NCORES, PAIRS, 128, HC * ND))
    gtT = np.ascontiguousarray(
        img_bf.reshape(NCORES, PAIRS, 2, R, HC, 128)
        .transpose(0, 1, 5, 4, 2, 3)
        .reshape(NCORES, PAIRS, 128, HC * NG))

    def w_chunks(W):  # oc-major lhsT layout: (oc, 128 h-part, hc*128 o)
        return np.ascontiguousarray(
            np.asarray(W, np.float32).T.reshape(HC, 128, HC, 128)
            .transpose(2, 1, 0, 3).reshape(HC, 128, HC * 128))

    w1d = _bf16(w_chunks(W_dns1))
    w4d = _bf16(w_chunks(W_img2))

    def w_rep(wv):  # (128 o-part, oc, 128 replicated)
        x = np.asarray(wv, np.float32).reshape(HC, 128).T
        return _bf16(np.ascontiguousarray(
            np.broadcast_to(x[:, :, None], (128, HC, 128))
            .reshape(128, HC * 128)))

    wr1 = w_rep(np.asarray(w_att1, np.float32)[H:])
    wr4 = w_rep(np.asarray(w_att2, np.float32)[H:])
    bc1 = np.ascontiguousarray(np.asarray(b_dns1, np.float32).reshape(HC, 128).T)
    bc4 = np.ascontiguousarray(np.asarray(b_img2, np.float32).reshape(HC, 128).T)

    in_maps = []
    for c in range(NCORES):
        in_maps.append({
            "dtT": dtT[c], "gtT": gtT[c],
            "w1d": w1d, "w4d": w4d, "wr1": wr1, "wr4": wr4,
            "bc1": bc1, "bc4": bc4,
        })
    return in_maps


def _unscramble_v(arr):
    # (128, BPC*HC) -> (BPC, H) with h = hc*128 + p
    return np.ascontiguousarray(
        np.asarray(arr, np.float32).reshape(128, BPC, HC)
        .transpose(1, 2, 0).reshape(BPC, H))


def run(inputs, trace=False):
    """Run on the 8 NeuronCores; returns (att_img, att_dns, exec_time_ns)."""
    from concourse.bass_utils import run_bass_kernel_spmd

    nc = _get_program()
    in_maps = _prepare_in_maps(
        inputs["dns_feature"], inputs["img_features"],
        inputs["W_dns1"], inputs["b_dns1"], inputs["W_img2"], inputs["b_img2"],
        inputs["w_att1"], inputs["w_att2"],
    )
    res = run_bass_kernel_spmd(nc, in_maps, core_ids=list(range(NCORES)),
                               trace=trace)
    v_dns, v_img = [], []
    for c in range(NCORES):
        vz = np.asarray(res.results[c]["v_img_z"], np.float32)
        z = vz[0, BPC * HC:]  # (2*BPC,) rows identical; row 0 suffices
        v_dns.append(_unscramble_v(res.results[c]["v_dns"]) / z[:BPC, None])
        v_img.append(_unscramble_v(vz[:, :BPC * HC]) / z[BPC:, None])
    v_dns = np.concatenate(v_dns, 0).astype(np.float32)
    v_img = np.concatenate(v_img, 0).astype(np.float32)
    att_dns = np.ascontiguousarray(
        np.broadcast_to(v_dns[:, None, :], (B, R, H)))
    att_img = np.ascontiguousarray(
        np.broadcast_to(v_img[:, None, :], (B, R, H)))
    return att_img, att_dns, res.exec_time_ns


def kernel(**inputs):
    att_img, att_dns, _ = run(inputs, trace=False)
    return att_img, att_dns


if __name__ == "__main__":
    prog = _get_program()
    print("program built + compiled OK")
